# revision 29
# baseline (speedup 1.0000x reference)
"""Trainium2 Bass kernel for nn_AIGStateEncoder (2-layer LSTM + linear head).

Data-parallel over batch: B=4096 rows split across 8 NeuronCores (512 each).
Per core the two LSTM layers are fused into one recurrence ("combined step"
s runs layer0 at t=s and layer1 at t=s-1), with the state kept transposed
(hidden units on SBUF partitions, batch on the free dimension) and the two
layers stacked on the 128 partitions: [layer0 (0:64); layer1 (64:128)].

Per combined step and batch-group (2 independent groups of 256 batch rows
hide each other's serial-dependency chain):
  - 8 matmuls (4 gates x 2 layers; K=65 for layer0 ([x_t; h0]), K=128 for
    layer1 ([h0; h1]); N=256) into per-gate PSUM banks,
  - 4 sigmoid/tanh ScalarE activations with per-partition bias (both layers
    in one [128, 256] instruction each),
  - cell/hidden updates on VectorE in bf16 (2x mode),
  - tanh(c) on ScalarE.
The per-step x row is DMA'd from DRAM into partition 64 of the layer0
moving operand; h0 is copied into it by VectorE.

Everything computes in bf16 (fp32 PSUM accumulation); measured end-to-end
relative error vs the fp32 reference is ~3e-3.
"""
import sys

if '/opt/trn_rl_repo' not in sys.path:
    sys.path.insert(0, '/opt/trn_rl_repo')

import numpy as np
import ml_dtypes

B, T, H = 4096, 256, 64
N_CORES = 8
B_LOC = B // N_CORES  # 512

GATE_ORDER = ("f", "g", "i", "o")   # emission order (c-critical gates first)
GCOL = {"i": 0, "f": 1, "g": 2, "o": 3}  # PyTorch gate order i,f,g,o


def _split_excess_waits(nc, limit=1):
    """The walrus build in this container accepts at most one sync wait per
    instruction.  Hoist excess waits onto NoOps inserted just before the
    instruction on the same engine (same-engine program order preserves the
    synchronization semantics)."""
    import concourse.mybir as mybir
    ctr = 0
    for f in nc.m.functions:
        for bb in f.blocks:
            il = bb.instructions
            i = 0
            while i < len(il):
                ins = il[i]
                si = ins.sync_info
                if si is not None and si.on_wait and len(si.on_wait) > limit:
                    waits = list(si.on_wait)
                    excess, keep = waits[:-limit], waits[-limit:]
                    while excess:
                        chunk, excess = excess[:limit], excess[limit:]
                        nop = mybir.InstNoOp(name=f"waitsplit_{ctr}", ins=[], outs=[])
                        ctr += 1
                        nop.engine = ins.engine
                        nop.sync_info = mybir.SyncInfo(on_wait=chunk, on_update=[])
                        il.insert(i, nop)
                        i += 1
                    ins.sync_info = mybir.SyncInfo(on_wait=keep,
                                                   on_update=list(si.on_update))
                i += 1


def _elide_redundant_waits(nc):
    """Drop semaphore waits that are already guaranteed.

    Two sources of dead waits: (1) repeated waits on the same monotone
    semaphore at a value a previous same-engine instruction already awaited
    (e.g. the constant DMAHW>=16 weight-preamble waits), and (2) waits whose
    value is transitively implied: if this engine earlier waited S_F>=v, the
    instruction that brought S_F to v had itself completed waits/updates that
    imply the current wait (e.g. an ACT's WAR wait on a DVE value from 3
    steps ago is implied by its own PE wait, because that matmul waited on
    DVE rhs-writes from 1 step ago).

    Safety: engines execute their instruction stream serially (wait ->
    execute -> update), so per-engine guarantees accumulate in program
    order.  Semaphores ever updated by DMA-completion (SP/DMA instructions)
    or by more than one engine are treated as async: waits on them still
    accumulate into guarantees (counters are monotone), but producer
    guarantees are never inherited through them (completion order vs
    emission order is not reliable).  Only plain 'sem-ge-imm' waits and
    'sem-inc'/'sem-add-imm' updates participate; anything else is kept
    verbatim.  Runs before _split_excess_waits so surviving multi-wait
    instructions still get their NoOp hoists.
    """
    import bisect
    import concourse.mybir as mybir

    ENGINE_SEM_PREFIXES = ("PE_", "Activation_", "DVE_", "Pool_")
    CONTROL_MARKERS = ("Semaphore", "Drain", "Branch", "Event", "Reset",
                       "Clear", "Halt", "Barrier", "NoOp")

    for f in nc.m.functions:
        for bb in f.blocks:
            sem_total = {}       # sem id -> emission-order running total
            producers = {}       # sem id -> ([totals], [guarantee dicts])
            sem_engines = {}     # sem id -> set of updating engine names
            async_sem = set()    # sem ids with unreliable update ordering
            eng_g = {}           # engine -> {sem id: guaranteed min value}
            for ins in bb.instructions:
                tname = type(ins).__name__
                if any(m in tname for m in CONTROL_MARKERS):
                    # sem manipulation / control flow: drop everything proven
                    eng_g = {}
                    producers = {}
                    async_sem.update(sem_total)
                    continue
                si = ins.sync_info
                eng = str(ins.engine)
                is_dma = "DMA" in tname.upper() or eng == "EngineType.SP"
                g = dict(eng_g.get(eng, ()))
                # Only DROP waits on the saturated Activation engine: its
                # waitsplit NoOps cost ~3.2% of the pacer queue.  PE/DVE keep
                # their waits (nearly free on queues with slack) -- removing
                # them lets the schedule drift into a bad phase equilibrium
                # (measured bistable 1008us/1209us with full elision).
                may_drop = eng in ("EngineType.Activation", "EngineType.PE")
                if si is not None and si.on_wait:
                    kept = []
                    for w in si.on_wait:
                        analyzable = (
                            w.sync_type == "semaphore"
                            and w.wait_mode == "sem-ge-imm"
                            and w.wait_reg is None
                            and isinstance(w.ant_name, str)
                            and w.ant_name.startswith(ENGINE_SEM_PREFIXES))
                        if not analyzable:
                            kept.append(w)
                            continue
                        sid, v = w.id, w.wait_value
                        if may_drop and sid not in async_sem and g.get(sid, 0) >= v:
                            continue  # provably satisfied already
                        kept.append(w)
                        if sid in async_sem:
                            continue
                        g[sid] = max(g.get(sid, 0), v)
                        if sid in producers:
                            totals, guards = producers[sid]
                            i = bisect.bisect_left(totals, v)
                            if i < len(totals):
                                for k, pv in guards[i].items():
                                    if k not in async_sem and pv > g.get(k, 0):
                                        g[k] = pv
                        continue
                    if len(kept) != len(si.on_wait):
                        ins.sync_info = mybir.SyncInfo(
                            on_wait=kept, on_update=list(si.on_update))
                        si = ins.sync_info
                # apply this instruction's updates to the model
                if si is not None:
                    for u in si.on_update:
                        if u.sync_type != "semaphore" or u.update_reg is not None:
                            continue
                        sid = u.id
                        if (u.update_mode not in ("sem-inc", "sem-add-imm")
                                or not isinstance(u.ant_name, str)
                                or not u.ant_name.startswith(ENGINE_SEM_PREFIXES)):
                            async_sem.add(sid)
                            continue
                        inc = u.update_value if u.update_value else 1
                        sem_total[sid] = sem_total.get(sid, 0) + inc
                        engs = sem_engines.setdefault(sid, set())
                        engs.add(eng)
                        if is_dma or len(engs) > 1:
                            async_sem.add(sid)
                            continue
                        g[sid] = sem_total[sid]
                        totals, guards = producers.setdefault(sid, ([], []))
                        totals.append(sem_total[sid])
                        guards.append(dict(g))
                eng_g[eng] = g


def _build_program(n_groups=2):
    import concourse.bass as bass
    import concourse.mybir as mybir
    from concourse.tile import TileContext

    BF16 = mybir.dt.bfloat16
    F32 = mybir.dt.float32
    AF = mybir.ActivationFunctionType
    OP = mybir.AluOpType
    GFUNC = {"i": AF.Sigmoid, "f": AF.Sigmoid, "g": AF.Tanh, "o": AF.Sigmoid}

    NG = n_groups
    NB = B_LOC // NG

    nc = bass.Bass()
    xT = nc.declare_dram_parameter("xT", [T, B_LOC], BF16, isOutput=False)
    w0 = nc.declare_dram_parameter("w0", [1 + H, 4 * H], BF16, isOutput=False)
    w1 = nc.declare_dram_parameter("w1", [2 * H, 4 * H], BF16, isOutput=False)
    bias = nc.declare_dram_parameter("bias", [2 * H, 4], F32, isOutput=False)
    wlin = nc.declare_dram_parameter("wlin", [H, H], BF16, isOutput=False)
    blin = nc.declare_dram_parameter("blin", [2 * H, H], F32, isOutput=False)
    out = nc.declare_dram_parameter("out", [B_LOC, H], F32, isOutput=True)

    with TileContext(nc) as tc:
        with (
            tc.tile_pool(name="const", bufs=1) as cpool,
            tc.tile_pool(name="state", bufs=1) as spool,
            tc.tile_pool(name="work", bufs=3) as wpool,
            tc.tile_pool(name="psum", bufs=1, space="PSUM") as ppool,
        ):
            w0_s = cpool.tile([1 + H, 4 * H], BF16, tag="w0", name="w0")
            nc.sync.dma_start(w0_s[:], w0[:])
            w1_s = cpool.tile([2 * H, 4 * H], BF16, tag="w1", name="w1")
            nc.sync.dma_start(w1_s[:], w1[:])
            bias_s = cpool.tile([2 * H, 4], F32, tag="bias", name="bias")
            nc.sync.dma_start(bias_s[:], bias[:])
            wlin_s = cpool.tile([H, H], BF16, tag="wlin", name="wlin")
            nc.sync.dma_start(wlin_s[:], wlin[:])
            blin_s = cpool.tile([2 * H, H], F32, tag="blin", name="blin")
            nc.sync.dma_start(blin_s[:], blin[:])

            rhsA = []  # [65, NB] : [h0 (0:64); x_t (64)]
            rhsB = []  # [128, NB]: [h0; h1]
            c_st = []  # [128, NB]: [c0; c1]
            h1f = []   # [64, NB] : final h1
            for g in range(NG):
                rhsA.append([spool.tile([1 + H, NB], BF16, tag=f"rhsA{g}_{p}",
                                        name=f"rhsA{g}_{p}") for p in range(2)])
                rhsB.append([spool.tile([2 * H, NB], BF16, tag=f"rhsB{g}_{p}",
                                        name=f"rhsB{g}_{p}") for p in range(2)])
                c_st.append(spool.tile([2 * H, NB], BF16, tag=f"c{g}", name=f"cst{g}"))
                h1f.append(spool.tile([H, NB], BF16, tag=f"h1f{g}", name=f"h1f{g}"))
                for p in range(2):
                    nc.gpsimd.memset(rhsA[g][p][:], 0.0)
                    nc.gpsimd.memset(rhsB[g][p][:], 0.0)
                nc.gpsimd.memset(c_st[g][:], 0.0)
                nc.sync.dma_start(rhsA[g][0][H:H + 1, :],
                                  xT[0:1, g * NB:(g + 1) * NB])

            def step(s):
                cur, nxt = s % 2, (s + 1) % 2
                l0 = s < T
                l1 = s >= 1
                lo = 0 if l0 else H
                hi = 2 * H if l1 else H
                for g in range(NG):
                    ps = {}
                    for gt in GATE_ORDER:
                        ps[gt] = ppool.tile([2 * H, NB], F32, tag=f"ps_{gt}{g}",
                                            name=f"ps_{gt}{g}")
                        col = GCOL[gt] * H
                        if l0:
                            nc.tensor.matmul(ps[gt][0:H, :], w0_s[:, col:col + H],
                                             rhsA[g][cur][:], start=True, stop=True)
                        if l1:
                            nc.tensor.matmul(ps[gt][H:2 * H, :], w1_s[:, col:col + H],
                                             rhsB[g][cur][:], start=True, stop=True)
                    act = {}
                    for gt in GATE_ORDER:
                        a = wpool.tile([2 * H, NB], BF16, tag=f"a_{gt}{g}",
                                       name=f"a_{gt}{g}")
                        nc.scalar.activation(a[lo:hi, :], ps[gt][lo:hi, :], GFUNC[gt],
                                             bias=bias_s[lo:hi, GCOL[gt]:GCOL[gt] + 1])
                        act[gt] = a
                    t1 = wpool.tile([2 * H, NB], BF16, tag=f"t1{g}", name=f"t1{g}")
                    nc.vector.tensor_tensor(t1[lo:hi, :], act["f"][lo:hi, :],
                                            c_st[g][lo:hi, :], op=OP.mult)
                    t2 = wpool.tile([2 * H, NB], BF16, tag=f"t2{g}", name=f"t2{g}")
                    nc.vector.tensor_tensor(t2[lo:hi, :], act["i"][lo:hi, :],
                                            act["g"][lo:hi, :], op=OP.mult)
                    nc.vector.tensor_tensor(c_st[g][lo:hi, :], t1[lo:hi, :],
                                            t2[lo:hi, :], op=OP.add)
                    tC = wpool.tile([2 * H, NB], BF16, tag=f"tC{g}", name=f"tC{g}")
                    nc.scalar.activation(tC[lo:hi, :], c_st[g][lo:hi, :], AF.Tanh)
                    if s < T:
                        # h0 directly into layer0's next rhs (parallel with the
                        # main h-update below, instead of a dependent copy)
                        nc.vector.tensor_tensor(rhsA[g][nxt][0:H, :],
                                                act["o"][0:H, :], tC[0:H, :],
                                                op=OP.mult)
                        nc.vector.tensor_tensor(rhsB[g][nxt][lo:hi, :],
                                                act["o"][lo:hi, :], tC[lo:hi, :],
                                                op=OP.mult)
                        if s + 1 < T:
                            nc.sync.dma_start(
                                rhsA[g][nxt][H:H + 1, :],
                                xT[s + 1:s + 2, g * NB:(g + 1) * NB])
                    else:
                        nc.vector.tensor_tensor(h1f[g][:], act["o"][H:2 * H, :],
                                                tC[H:2 * H, :], op=OP.mult)

            for s in range(T + 1):
                step(s)

            # final linear: out[b, :] = h1f.T @ wlin + blin
            for g in range(NG):
                for blk in range(NB // 128):
                    psl = ppool.tile([128, H], F32, tag="ps_f0", name="psl")
                    nc.tensor.matmul(psl[:], h1f[g][:, blk * 128:(blk + 1) * 128],
                                     wlin_s[:], start=True, stop=True)
                    ob = wpool.tile([128, H], F32, tag="ob", name="ob")
                    import concourse.mybir as _mb
                    nc.vector.scalar_tensor_tensor(ob[:], psl[:], 1.0,
                                                   blin_s[0:128, :],
                                                   op0=OP.mult, op1=OP.add)
                    row0 = g * NB + blk * 128
                    nc.sync.dma_start(out[row0:row0 + 128, :], ob[:])

    _elide_redundant_waits(nc)
    _split_excess_waits(nc, limit=1)
    return nc


def _prep_inputs(inputs):
    bf = ml_dtypes.bfloat16
    recipe = np.ascontiguousarray(np.asarray(inputs["recipe"], np.float32).reshape(B, T))
    w0 = np.concatenate([np.asarray(inputs["W_hh0"]).T,
                         np.asarray(inputs["W_ih0"]).T], axis=0).astype(bf)
    w1 = np.concatenate([np.asarray(inputs["W_ih1"]).T,
                         np.asarray(inputs["W_hh1"]).T], axis=0).astype(bf)
    b0 = (np.asarray(inputs["b_ih0"]) + np.asarray(inputs["b_hh0"])).astype(np.float32)
    b1 = (np.asarray(inputs["b_ih1"]) + np.asarray(inputs["b_hh1"])).astype(np.float32)
    bias = np.zeros((2 * H, 4), np.float32)
    for gi in range(4):
        bias[0:H, gi] = b0[gi * H:(gi + 1) * H]
        bias[H:2 * H, gi] = b1[gi * H:(gi + 1) * H]
    wlin = np.ascontiguousarray(np.asarray(inputs["W_lin"]).T).astype(bf)
    blin = np.tile(np.asarray(inputs["b_lin"], np.float32), (2 * H, 1))
    in_maps = []
    for i in range(N_CORES):
        shard = recipe[i * B_LOC:(i + 1) * B_LOC]
        xTs = np.ascontiguousarray(shard.T).astype(bf)
        in_maps.append({"xT": xTs, "w0": w0, "w1": w1, "bias": bias,
                        "wlin": wlin, "blin": blin})
    return in_maps


_PROGRAM = []


def _run(inputs, trace=False):
    from concourse.bass_utils import run_bass_kernel_spmd
    if not _PROGRAM:
        _PROGRAM.append(_build_program())
    nc = _PROGRAM[0]
    in_maps = _prep_inputs(inputs)
    last_err = None
    for attempt in range(3):
        try:
            res = run_bass_kernel_spmd(nc, in_maps,
                                       core_ids=list(range(N_CORES)), trace=trace)
            outs = [np.asarray(res.results[i]["out"]) for i in range(N_CORES)]
            return np.concatenate(outs, axis=0), res
        except Exception as e:  # transient first-exec device faults: retry
            last_err = e
    raise last_err


def kernel(**inputs):
    full, _ = _run(inputs, trace=False)
    return full.astype(np.float32)



# revision 30
# speedup vs baseline: 1.0496x; 1.0496x over previous
"""Trainium2 Bass kernel for nn_AIGStateEncoder (2-layer LSTM + linear head).

Data-parallel over batch: B=4096 rows split across 8 NeuronCores (512 each).
Per core the two LSTM layers are fused into one recurrence ("combined step"
s runs layer0 at t=s and layer1 at t=s-1), with the state kept transposed
(hidden units on SBUF partitions, batch on the free dimension) and the two
layers stacked on the 128 partitions: [layer0 (0:64); layer1 (64:128)].

Per combined step and batch-group (2 independent groups of 256 batch rows
hide each other's serial-dependency chain):
  - 8 matmuls (4 gates x 2 layers; K=65 for layer0 ([x_t; h0]), K=128 for
    layer1 ([h0; h1]); N=256) into per-gate PSUM banks,
  - 4 sigmoid/tanh ScalarE activations with per-partition bias (both layers
    in one [128, 256] instruction each),
  - cell/hidden updates on VectorE in bf16 (2x mode),
  - tanh(c) on ScalarE.
The per-step x row is DMA'd from DRAM into partition 64 of the layer0
moving operand; h0 is copied into it by VectorE.

Everything computes in bf16 (fp32 PSUM accumulation); measured end-to-end
relative error vs the fp32 reference is ~3e-3.
"""
import sys

if '/opt/trn_rl_repo' not in sys.path:
    sys.path.insert(0, '/opt/trn_rl_repo')

import numpy as np
import ml_dtypes

B, T, H = 4096, 256, 64
N_CORES = 8
B_LOC = B // N_CORES  # 512

GATE_ORDER = ("f", "g", "i", "o")   # emission order (c-critical gates first)
GCOL = {"i": 0, "f": 1, "g": 2, "o": 3}  # PyTorch gate order i,f,g,o


def _split_excess_waits(nc, limit=1):
    """The walrus build in this container accepts at most one sync wait per
    instruction.  Hoist excess waits onto NoOps inserted just before the
    instruction on the same engine (same-engine program order preserves the
    synchronization semantics)."""
    import concourse.mybir as mybir
    ctr = 0
    for f in nc.m.functions:
        for bb in f.blocks:
            il = bb.instructions
            i = 0
            while i < len(il):
                ins = il[i]
                si = ins.sync_info
                if si is not None and si.on_wait and len(si.on_wait) > limit:
                    waits = list(si.on_wait)
                    excess, keep = waits[:-limit], waits[-limit:]
                    while excess:
                        chunk, excess = excess[:limit], excess[limit:]
                        nop = mybir.InstNoOp(name=f"waitsplit_{ctr}", ins=[], outs=[])
                        ctr += 1
                        nop.engine = ins.engine
                        nop.sync_info = mybir.SyncInfo(on_wait=chunk, on_update=[])
                        il.insert(i, nop)
                        i += 1
                    ins.sync_info = mybir.SyncInfo(on_wait=keep,
                                                   on_update=list(si.on_update))
                i += 1


def _elide_redundant_waits(nc):
    """Drop semaphore waits that are already guaranteed.

    Two sources of dead waits: (1) repeated waits on the same monotone
    semaphore at a value a previous same-engine instruction already awaited
    (e.g. the constant DMAHW>=16 weight-preamble waits), and (2) waits whose
    value is transitively implied: if this engine earlier waited S_F>=v, the
    instruction that brought S_F to v had itself completed waits/updates that
    imply the current wait (e.g. an ACT's WAR wait on a DVE value from 3
    steps ago is implied by its own PE wait, because that matmul waited on
    DVE rhs-writes from 1 step ago).

    Safety: engines execute their instruction stream serially (wait ->
    execute -> update), so per-engine guarantees accumulate in program
    order.  Semaphores ever updated by DMA-completion (SP/DMA instructions)
    or by more than one engine are treated as async: waits on them still
    accumulate into guarantees (counters are monotone), but producer
    guarantees are never inherited through them (completion order vs
    emission order is not reliable).  Only plain 'sem-ge-imm' waits and
    'sem-inc'/'sem-add-imm' updates participate; anything else is kept
    verbatim.  Runs before _split_excess_waits so surviving multi-wait
    instructions still get their NoOp hoists.
    """
    import bisect
    import concourse.mybir as mybir

    ENGINE_SEM_PREFIXES = ("PE_", "Activation_", "DVE_", "Pool_")
    CONTROL_MARKERS = ("Semaphore", "Drain", "Branch", "Event", "Reset",
                       "Clear", "Halt", "Barrier", "NoOp")

    for f in nc.m.functions:
        for bb in f.blocks:
            sem_total = {}       # sem id -> emission-order running total
            producers = {}       # sem id -> ([totals], [guarantee dicts])
            sem_engines = {}     # sem id -> set of updating engine names
            async_sem = set()    # sem ids with unreliable update ordering
            eng_g = {}           # engine -> {sem id: guaranteed min value}
            for ins in bb.instructions:
                tname = type(ins).__name__
                if any(m in tname for m in CONTROL_MARKERS):
                    # sem manipulation / control flow: drop everything proven
                    eng_g = {}
                    producers = {}
                    async_sem.update(sem_total)
                    continue
                si = ins.sync_info
                eng = str(ins.engine)
                is_dma = "DMA" in tname.upper() or eng == "EngineType.SP"
                g = dict(eng_g.get(eng, ()))
                # Only DROP waits on the saturated Activation engine: its
                # waitsplit NoOps cost ~3.2% of the pacer queue.  PE/DVE keep
                # their waits (nearly free on queues with slack) -- removing
                # them lets the schedule drift into a bad phase equilibrium
                # (measured bistable 1008us/1209us with full elision).
                may_drop = eng in ("EngineType.Activation", "EngineType.DVE")
                if si is not None and si.on_wait:
                    kept = []
                    for w in si.on_wait:
                        analyzable = (
                            w.sync_type == "semaphore"
                            and w.wait_mode == "sem-ge-imm"
                            and w.wait_reg is None
                            and isinstance(w.ant_name, str)
                            and w.ant_name.startswith(ENGINE_SEM_PREFIXES))
                        if not analyzable:
                            kept.append(w)
                            continue
                        sid, v = w.id, w.wait_value
                        if may_drop and sid not in async_sem and g.get(sid, 0) >= v:
                            continue  # provably satisfied already
                        kept.append(w)
                        if sid in async_sem:
                            continue
                        g[sid] = max(g.get(sid, 0), v)
                        if sid in producers:
                            totals, guards = producers[sid]
                            i = bisect.bisect_left(totals, v)
                            if i < len(totals):
                                for k, pv in guards[i].items():
                                    if k not in async_sem and pv > g.get(k, 0):
                                        g[k] = pv
                        continue
                    if len(kept) != len(si.on_wait):
                        ins.sync_info = mybir.SyncInfo(
                            on_wait=kept, on_update=list(si.on_update))
                        si = ins.sync_info
                # apply this instruction's updates to the model
                if si is not None:
                    for u in si.on_update:
                        if u.sync_type != "semaphore" or u.update_reg is not None:
                            continue
                        sid = u.id
                        if (u.update_mode not in ("sem-inc", "sem-add-imm")
                                or not isinstance(u.ant_name, str)
                                or not u.ant_name.startswith(ENGINE_SEM_PREFIXES)):
                            async_sem.add(sid)
                            continue
                        inc = u.update_value if u.update_value else 1
                        sem_total[sid] = sem_total.get(sid, 0) + inc
                        engs = sem_engines.setdefault(sid, set())
                        engs.add(eng)
                        if is_dma or len(engs) > 1:
                            async_sem.add(sid)
                            continue
                        g[sid] = sem_total[sid]
                        totals, guards = producers.setdefault(sid, ([], []))
                        totals.append(sem_total[sid])
                        guards.append(dict(g))
                eng_g[eng] = g


def _build_program(n_groups=2):
    import concourse.bass as bass
    import concourse.mybir as mybir
    from concourse.tile import TileContext

    BF16 = mybir.dt.bfloat16
    F32 = mybir.dt.float32
    AF = mybir.ActivationFunctionType
    OP = mybir.AluOpType
    GFUNC = {"i": AF.Sigmoid, "f": AF.Sigmoid, "g": AF.Tanh, "o": AF.Sigmoid}

    NG = n_groups
    NB = B_LOC // NG

    nc = bass.Bass()
    xT = nc.declare_dram_parameter("xT", [T, B_LOC], BF16, isOutput=False)
    w0 = nc.declare_dram_parameter("w0", [1 + H, 4 * H], BF16, isOutput=False)
    w1 = nc.declare_dram_parameter("w1", [2 * H, 4 * H], BF16, isOutput=False)
    bias = nc.declare_dram_parameter("bias", [2 * H, 4], F32, isOutput=False)
    wlin = nc.declare_dram_parameter("wlin", [H, H], BF16, isOutput=False)
    blin = nc.declare_dram_parameter("blin", [2 * H, H], F32, isOutput=False)
    out = nc.declare_dram_parameter("out", [B_LOC, H], F32, isOutput=True)

    with TileContext(nc) as tc:
        with (
            tc.tile_pool(name="const", bufs=1) as cpool,
            tc.tile_pool(name="state", bufs=1) as spool,
            tc.tile_pool(name="work", bufs=3) as wpool,
            tc.tile_pool(name="psum", bufs=1, space="PSUM") as ppool,
        ):
            w0_s = cpool.tile([1 + H, 4 * H], BF16, tag="w0", name="w0")
            nc.sync.dma_start(w0_s[:], w0[:])
            w1_s = cpool.tile([2 * H, 4 * H], BF16, tag="w1", name="w1")
            nc.sync.dma_start(w1_s[:], w1[:])
            bias_s = cpool.tile([2 * H, 4], F32, tag="bias", name="bias")
            nc.sync.dma_start(bias_s[:], bias[:])
            wlin_s = cpool.tile([H, H], BF16, tag="wlin", name="wlin")
            nc.sync.dma_start(wlin_s[:], wlin[:])
            blin_s = cpool.tile([2 * H, H], F32, tag="blin", name="blin")
            nc.sync.dma_start(blin_s[:], blin[:])

            rhsA = []  # [65, NB] : [h0 (0:64); x_t (64)]
            rhsB = []  # [128, NB]: [h0; h1]
            c_st = []  # [128, NB]: [c0; c1]
            h1f = []   # [64, NB] : final h1
            for g in range(NG):
                rhsA.append([spool.tile([1 + H, NB], BF16, tag=f"rhsA{g}_{p}",
                                        name=f"rhsA{g}_{p}") for p in range(2)])
                rhsB.append([spool.tile([2 * H, NB], BF16, tag=f"rhsB{g}_{p}",
                                        name=f"rhsB{g}_{p}") for p in range(2)])
                c_st.append(spool.tile([2 * H, NB], BF16, tag=f"c{g}", name=f"cst{g}"))
                h1f.append(spool.tile([H, NB], BF16, tag=f"h1f{g}", name=f"h1f{g}"))
                for p in range(2):
                    nc.gpsimd.memset(rhsA[g][p][:], 0.0)
                    nc.gpsimd.memset(rhsB[g][p][:], 0.0)
                nc.gpsimd.memset(c_st[g][:], 0.0)
                nc.sync.dma_start(rhsA[g][0][H:H + 1, :],
                                  xT[0:1, g * NB:(g + 1) * NB])

            def step(s):
                cur, nxt = s % 2, (s + 1) % 2
                l0 = s < T
                l1 = s >= 1
                lo = 0 if l0 else H
                hi = 2 * H if l1 else H
                for g in range(NG):
                    ps = {}
                    for gt in GATE_ORDER:
                        ps[gt] = ppool.tile([2 * H, NB], F32, tag=f"ps_{gt}{g}",
                                            name=f"ps_{gt}{g}")
                        col = GCOL[gt] * H
                        if l0:
                            nc.tensor.matmul(ps[gt][0:H, :], w0_s[:, col:col + H],
                                             rhsA[g][cur][:], start=True, stop=True)
                        if l1:
                            nc.tensor.matmul(ps[gt][H:2 * H, :], w1_s[:, col:col + H],
                                             rhsB[g][cur][:], start=True, stop=True)
                    act = {}
                    for gt in GATE_ORDER:
                        a = wpool.tile([2 * H, NB], BF16, tag=f"a_{gt}{g}",
                                       name=f"a_{gt}{g}")
                        nc.scalar.activation(a[lo:hi, :], ps[gt][lo:hi, :], GFUNC[gt],
                                             bias=bias_s[lo:hi, GCOL[gt]:GCOL[gt] + 1])
                        act[gt] = a
                    t1 = wpool.tile([2 * H, NB], BF16, tag=f"t1{g}", name=f"t1{g}")
                    nc.vector.tensor_tensor(t1[lo:hi, :], act["f"][lo:hi, :],
                                            c_st[g][lo:hi, :], op=OP.mult)
                    t2 = wpool.tile([2 * H, NB], BF16, tag=f"t2{g}", name=f"t2{g}")
                    nc.vector.tensor_tensor(t2[lo:hi, :], act["i"][lo:hi, :],
                                            act["g"][lo:hi, :], op=OP.mult)
                    nc.vector.tensor_tensor(c_st[g][lo:hi, :], t1[lo:hi, :],
                                            t2[lo:hi, :], op=OP.add)
                    tC = wpool.tile([2 * H, NB], BF16, tag=f"tC{g}", name=f"tC{g}")
                    nc.scalar.activation(tC[lo:hi, :], c_st[g][lo:hi, :], AF.Tanh)
                    if s < T:
                        # h0 directly into layer0's next rhs (parallel with the
                        # main h-update below, instead of a dependent copy)
                        nc.vector.tensor_tensor(rhsA[g][nxt][0:H, :],
                                                act["o"][0:H, :], tC[0:H, :],
                                                op=OP.mult)
                        nc.vector.tensor_tensor(rhsB[g][nxt][lo:hi, :],
                                                act["o"][lo:hi, :], tC[lo:hi, :],
                                                op=OP.mult)
                        if s + 1 < T:
                            nc.sync.dma_start(
                                rhsA[g][nxt][H:H + 1, :],
                                xT[s + 1:s + 2, g * NB:(g + 1) * NB])
                    else:
                        nc.vector.tensor_tensor(h1f[g][:], act["o"][H:2 * H, :],
                                                tC[H:2 * H, :], op=OP.mult)

            for s in range(T + 1):
                step(s)

            # final linear: out[b, :] = h1f.T @ wlin + blin
            for g in range(NG):
                for blk in range(NB // 128):
                    psl = ppool.tile([128, H], F32, tag="ps_f0", name="psl")
                    nc.tensor.matmul(psl[:], h1f[g][:, blk * 128:(blk + 1) * 128],
                                     wlin_s[:], start=True, stop=True)
                    ob = wpool.tile([128, H], F32, tag="ob", name="ob")
                    import concourse.mybir as _mb
                    nc.vector.scalar_tensor_tensor(ob[:], psl[:], 1.0,
                                                   blin_s[0:128, :],
                                                   op0=OP.mult, op1=OP.add)
                    row0 = g * NB + blk * 128
                    nc.sync.dma_start(out[row0:row0 + 128, :], ob[:])

    _elide_redundant_waits(nc)
    _split_excess_waits(nc, limit=1)
    return nc


def _prep_inputs(inputs):
    bf = ml_dtypes.bfloat16
    recipe = np.ascontiguousarray(np.asarray(inputs["recipe"], np.float32).reshape(B, T))
    w0 = np.concatenate([np.asarray(inputs["W_hh0"]).T,
                         np.asarray(inputs["W_ih0"]).T], axis=0).astype(bf)
    w1 = np.concatenate([np.asarray(inputs["W_ih1"]).T,
                         np.asarray(inputs["W_hh1"]).T], axis=0).astype(bf)
    b0 = (np.asarray(inputs["b_ih0"]) + np.asarray(inputs["b_hh0"])).astype(np.float32)
    b1 = (np.asarray(inputs["b_ih1"]) + np.asarray(inputs["b_hh1"])).astype(np.float32)
    bias = np.zeros((2 * H, 4), np.float32)
    for gi in range(4):
        bias[0:H, gi] = b0[gi * H:(gi + 1) * H]
        bias[H:2 * H, gi] = b1[gi * H:(gi + 1) * H]
    wlin = np.ascontiguousarray(np.asarray(inputs["W_lin"]).T).astype(bf)
    blin = np.tile(np.asarray(inputs["b_lin"], np.float32), (2 * H, 1))
    in_maps = []
    for i in range(N_CORES):
        shard = recipe[i * B_LOC:(i + 1) * B_LOC]
        xTs = np.ascontiguousarray(shard.T).astype(bf)
        in_maps.append({"xT": xTs, "w0": w0, "w1": w1, "bias": bias,
                        "wlin": wlin, "blin": blin})
    return in_maps


_PROGRAM = []


def _run(inputs, trace=False):
    from concourse.bass_utils import run_bass_kernel_spmd
    if not _PROGRAM:
        _PROGRAM.append(_build_program())
    nc = _PROGRAM[0]
    in_maps = _prep_inputs(inputs)
    last_err = None
    for attempt in range(3):
        try:
            res = run_bass_kernel_spmd(nc, in_maps,
                                       core_ids=list(range(N_CORES)), trace=trace)
            outs = [np.asarray(res.results[i]["out"]) for i in range(N_CORES)]
            return np.concatenate(outs, axis=0), res
        except Exception as e:  # transient first-exec device faults: retry
            last_err = e
    raise last_err


def kernel(**inputs):
    full, _ = _run(inputs, trace=False)
    return full.astype(np.float32)

